# revision 14
# baseline (speedup 1.0000x reference)
"""GRU4Rec Trainium2 kernel: 8-core SPMD.

Sharding: data-parallel over batch for embedding gather + input-gate matmuls +
GRU recurrence (32 sequences/core); vocab-sharded tied-embedding logits with an
on-device AllGather of the final hidden state.

Layout: "transposed" — hidden dim on partitions, batch on the free dim.
  - gates psum tile [128, slot, 32]: slots z0 z1 r0 r1 (zr bank) / g0 g1 (g bank)
  - per-step x-side z/r gates injected into PSUM via an identity-matmul with
    stop=True (keeps the serial elementwise chain off the Vector engine)
  - padding mask folded in as +BIG on the z-gate via a K=1 matmul of a 0/1 row
  - h kept fp32 (hf) with the fp32 update emitted AFTER the next step's
    matmuls (off the critical path); bf16 copy (hb) feeds the recurrence
  - phase-1 gathers read a bf16 copy of the embedding table and each chunk's
    work is spread across its 8 steps; PSUM->SBUF moves distributed over
    Scalar/Vector/GpSimd
  - logits: vocab slice preloaded to SBUF during the recurrence, bf16 output
    upconverted on host
"""

import numpy as np
import ml_dtypes

B, T, H, V = 256, 200, 256, 50000
NCORES = 8
BL = B // NCORES          # 32 sequences per core
NTOK = BL * T             # 6400 tokens per core
VS = 6250                 # vocab stride per core
VSC = VS + 1              # per-core logits width (overlap of 1, core 7 owns +1 row)
CHUNK_T = 8               # timesteps per phase-1 chunk
NCHUNK = T // CHUNK_T     # 25
CTOK = BL * CHUNK_T       # 256 tokens per chunk
BIGMASK = 60.0            # sigmoid(x + 60) == 1.0 in fp32

_cache = {}


def _build_nc(with_bias=False, debug=False):
    import concourse.bass as bass
    import concourse.mybir as mybir
    import concourse.tile as tile
    from concourse import bacc
    from concourse.bass import IndirectOffsetOnAxis

    f32 = mybir.dt.float32
    bf16 = mybir.dt.bfloat16
    i32 = mybir.dt.int32
    AF = mybir.ActivationFunctionType
    OP = mybir.AluOpType

    nc = bacc.Bacc(None, target_bir_lowering=False, debug=False, num_devices=NCORES)

    ids_d = nc.dram_tensor("ids", [NTOK, 1], i32, kind="ExternalInput")
    maskr_d = nc.dram_tensor("maskrow", [1, NTOK], bf16, kind="ExternalInput")
    embbf_d = nc.dram_tensor("embbf", [V + 1, H], bf16, kind="ExternalInput")
    embt_d = nc.dram_tensor("embt", [H, VSC], bf16, kind="ExternalInput")
    wih_d = nc.dram_tensor("wih", [H, 3 * H], bf16, kind="ExternalInput")
    whh_d = nc.dram_tensor("whh", [H, 3 * H], bf16, kind="ExternalInput")
    idm_d = nc.dram_tensor("idm", [128, 128], f32, kind="ExternalInput")
    if with_bias:
        biasf_d = nc.dram_tensor("biasf", [3 * H, 1], f32, kind="ExternalInput")
        bhhg_d = nc.dram_tensor("bhhg", [H, 1], f32, kind="ExternalInput")
    out_d = nc.dram_tensor("out", [B, VSC], f32, kind="ExternalOutput")
    if debug:
        dgzr_d = nc.dram_tensor("dgzr", [128, T, 4, 32], bf16, kind="ExternalOutput")
        dgg_d = nc.dram_tensor("dgg", [128, T, 2, 32], f32, kind="ExternalOutput")
        dh_d = nc.dram_tensor("dh", [128, 2, 32], bf16, kind="ExternalOutput")
        dxt_d = nc.dram_tensor("dxt", [128, 2, CTOK], bf16, kind="ExternalOutput")
        dstep_d = nc.dram_tensor("dstep", [2, 128, 12, 32], f32,
                                 kind="ExternalOutput")
        dpsum_d = nc.dram_tensor("dpsum", [128, 4, 32], f32,
                                 kind="ExternalOutput")

    with tile.TileContext(nc) as tc:
        with (
            tc.tile_pool(name="const", bufs=1) as const,
            tc.tile_pool(name="gstore", bufs=1) as gstore,
            tc.tile_pool(name="gin", bufs=3) as gin,
            tc.tile_pool(name="xtp", bufs=2) as xtp,
            tc.tile_pool(name="ew", bufs=2) as ew,
            tc.tile_pool(name="hst", bufs=2) as hst,
            tc.tile_pool(name="ldram", bufs=1, space="DRAM") as ldram,
            tc.tile_pool(name="ptr", bufs=2, space="PSUM") as ptr,
            tc.tile_pool(name="pg1", bufs=2, space="PSUM") as pg1,
            tc.tile_pool(name="pzr", bufs=2, space="PSUM") as pzr,
            tc.tile_pool(name="pgg", bufs=2, space="PSUM") as pgg,
        ):
            # ---- constants / weights ----
            wih_sb = const.tile([128, 2 * 3 * H], bf16)
            whh_sb = const.tile([128, 2 * 3 * H], bf16)
            for k in range(2):
                nc.sync.dma_start(wih_sb[:, k * 768:(k + 1) * 768],
                                  wih_d[k * 128:(k + 1) * 128, :])
                nc.sync.dma_start(whh_sb[:, k * 768:(k + 1) * 768],
                                  whh_d[k * 128:(k + 1) * 128, :])
            ident_f = const.tile([128, 128], f32)
            nc.sync.dma_start(ident_f[:, :], idm_d[:, :])
            ident = const.tile([128, 128], bf16)
            nc.vector.tensor_copy(ident[:, :], ident_f[:, :])
            wbig = const.tile([1, 128], bf16)
            nc.vector.memset(wbig[:, :], BIGMASK)
            maskr = const.tile([1, NTOK], bf16)
            nc.sync.dma_start(maskr[:, :], maskr_d[:, :])
            # logits vocab slice: preload during recurrence (25KB/partition)
            embt_sb = const.tile([128, 2, VSC], bf16)
            for k in range(2):
                nc.sync.dma_start(embt_sb[:, k, :],
                                  embt_d[k * 128:(k + 1) * 128, :])
            if with_bias:
                bias_sb = const.tile([128, 6], f32)
                nc.sync.dma_start(bias_sb[:, :],
                                  biasf_d.rearrange("(m p) o -> p (m o)", p=128))
                bhhg_sb = const.tile([128, 2], f32)
                nc.sync.dma_start(bhhg_sb[:, :],
                                  bhhg_d.rearrange("(g p) o -> p (g o)", p=128))

            # persistent gate stores
            gzr = gstore.tile([128, T, 4, 32], bf16)   # x-side z,r (+bias, +mask)
            gg = gstore.tile([128, T, 2, 32], f32)     # x-side h-candidate (+b_ih)

            # ---- phase-1, one chunk = 8 pieces spread over 8 steps ----
            # piece 0/1: gather 128 tokens (bf16); 2/3: transpose them;
            # 4/5/6: gate matmuls + PSUM->SBUF moves (2 gate slots per piece)
            chunk_state = {}
            deferred = []

            def emit_piece(c, j):
                tok0 = c * CTOK
                st = chunk_state.setdefault(c, {})
                if j in (0, 1):
                    idt = gin.tile([128, 1], i32, tag="idt")
                    nc.sync.dma_start(idt[:, :],
                                      ids_d[tok0 + j * 128: tok0 + (j + 1) * 128, :])
                    xg = gin.tile([128, H], bf16, tag="xg")
                    nc.gpsimd.indirect_dma_start(
                        out=xg[:, :], out_offset=None, in_=embbf_d[:, :],
                        in_offset=IndirectOffsetOnAxis(ap=idt[:, :1], axis=0))
                    st[j] = xg
                elif j in (2, 3):
                    tt = j - 2
                    if tt == 0:
                        st["xtc"] = xtp.tile([128, 2, CTOK], bf16, tag="xtc",
                                             name="xtc")
                    xg = st.pop(tt)
                    for hk in range(2):
                        pt = ptr.tile([128, 128], bf16, tag="pt")
                        nc.tensor.transpose(pt[:, :], xg[:, hk * 128:(hk + 1) * 128],
                                            ident[:, :])
                        dstx = st["xtc"][:, hk, tt * 128:(tt + 1) * 128]
                        if hk == 0:
                            nc.vector.tensor_copy(dstx, pt[:, :])
                        else:
                            nc.scalar.copy(dstx, pt[:, :])
                elif j in (4, 5, 6):
                    xtc = st["xtc"]
                    csl = slice(c * CHUNK_T, (c + 1) * CHUNK_T)
                    for mi, m in enumerate((2 * (j - 4), 2 * (j - 4) + 1)):
                        pg = pg1.tile([128, CHUNK_T, 32], f32, tag="pg")
                        for k in range(2):
                            nc.tensor.matmul(
                                pg[:, :, :],
                                wih_sb[:, k * 768 + m * 128: k * 768 + (m + 1) * 128],
                                xtc[:, k, :],
                                start=(k == 0), stop=(k == 1 and m >= 2))
                        if m < 2:  # z-gate slots: add BIGMASK * is_padded(token)
                            nc.tensor.matmul(
                                pg[:, :, :], wbig[:1, :],
                                maskr[:1, tok0: tok0 + CTOK],
                                start=False, stop=True)
                        dst = gzr[:, csl, m, :] if m < 4 else gg[:, csl, m - 4, :]
                        if with_bias:
                            nc.scalar.add(dst, pg[:, :, :], bias_sb[:, m:m + 1])
                        elif mi == 0:
                            nc.scalar.copy(dst, pg[:, :, :])
                        else:
                            nc.vector.tensor_copy(dst, pg[:, :, :])
                    if j == 6:
                        if debug and c == 0:
                            nc.sync.dma_start(dxt_d[:, :, :], xtc[:, :, :])
                        chunk_state.pop(c)

            # ---- recurrence ----
            hf = hst.tile([128, 2, 32], f32, tag="hf")
            hb = hst.tile([128, 2, 32], bf16, tag="hb")
            nc.vector.memset(hf[:, :, :], 0.0)
            nc.vector.memset(hb[:, :, :], 0.0)

            for c in range(2):
                for j in range(7):
                    emit_piece(c, j)
                    for fn in deferred:
                        fn()
                    deferred.clear()

            prev = None  # (us, ws, hf_prev) of step t-1, fp32 update deferred
            for t in range(T):
                cn = t // CHUNK_T + 2
                if cn < NCHUNK:
                    emit_piece(cn, t % CHUNK_T)

                przr = pzr.tile([128, 4, 32], f32, tag="przr")
                prg = pgg.tile([128, 2, 32], f32, tag="prg")
                for s in range(4):
                    for k in range(2):
                        nc.tensor.matmul(
                            przr[:, s, :],
                            whh_sb[:, k * 768 + s * 128: k * 768 + (s + 1) * 128],
                            hb[:, k, :], start=(k == 0), stop=False)
                    # x-side z/r gate (incl. mask) injected via identity-matmul
                    nc.tensor.matmul(przr[:, s, :], ident[:, :],
                                     gzr[:, t, s, :], start=False, stop=True)
                for s in range(2):
                    for k in range(2):
                        nc.tensor.matmul(
                            prg[:, s, :],
                            whh_sb[:, k * 768 + (4 + s) * 128: k * 768 + (5 + s) * 128],
                            hb[:, k, :], start=(k == 0), stop=(k == 1))

                # deferred fp32 h update for step t-1 (off the critical path:
                # emitted after this step's matmuls, needed only by us(t))
                if prev is not None:
                    hf = hst.tile([128, 2, 32], f32, tag="hf")
                    nc.gpsimd.tensor_tensor(hf[:, :, :], prev[0], prev[1],
                                            op=OP.add)

                if debug and t == 1:
                    pcp = ew.tile([128, 4, 32], f32, tag="pcp")
                    nc.vector.tensor_copy(pcp[:, :, :], przr[:, :, :])
                    nc.sync.dma_start(dpsum_d[:, :, :], pcp[:, :, :])
                zr = ew.tile([128, 4, 32], f32, tag="zr")
                nc.scalar.activation(zr[:, :, :], przr[:, :, :], AF.Sigmoid)
                qs = ew.tile([128, 2, 32], f32, tag="qs")
                if with_bias:
                    for s in range(2):
                        nc.vector.scalar_tensor_tensor(
                            qs[:, s, :], prg[:, s, :], bhhg_sb[:, s:s + 1],
                            zr[:, 2 + s, :], op0=OP.add, op1=OP.mult)
                else:
                    nc.vector.tensor_tensor(qs[:, :, :], prg[:, :, :],
                                            zr[:, 2:4, :], op=OP.mult)
                q2 = ew.tile([128, 2, 32], f32, tag="q2")
                nc.vector.tensor_tensor(q2[:, :, :], qs[:, :, :], gg[:, t, :, :],
                                        op=OP.add)
                zc = ew.tile([128, 2, 32], f32, tag="zc")
                nc.gpsimd.tensor_scalar(zc[:, :, :], zr[:, 0:2, :], -1.0, 1.0,
                                        OP.mult, OP.add)
                us = ew.tile([128, 2, 32], f32, tag="us")
                nc.gpsimd.tensor_tensor(us[:, :, :], zr[:, 0:2, :], hf[:, :, :],
                                        op=OP.mult)
                hh = ew.tile([128, 2, 32], f32, tag="hh")
                nc.scalar.activation(hh[:, :, :], q2[:, :, :], AF.Tanh)
                ws = ew.tile([128, 2, 32], f32, tag="ws")
                nc.vector.tensor_tensor(ws[:, :, :], zc[:, :, :], hh[:, :, :],
                                        op=OP.mult)
                hb = hst.tile([128, 2, 32], bf16, tag="hb")
                nc.vector.tensor_tensor(hb[:, :, :], us[:, :, :], ws[:, :, :],
                                        op=OP.add)
                prev = (us[:, :, :], ws[:, :, :])
                if debug and t < 2:
                    hfd = ew.tile([128, 2, 32], f32, tag="hfd")
                    nc.gpsimd.tensor_tensor(hfd[:, :, :], us[:, :, :],
                                            ws[:, :, :], op=OP.add)
                    nc.sync.dma_start(dstep_d[t, :, 0:2, :], hfd[:, :, :])
                    nc.sync.dma_start(dstep_d[t, :, 2:6, :], zr[:, :, :])
                    nc.sync.dma_start(dstep_d[t, :, 6:8, :], hh[:, :, :])
                    nc.sync.dma_start(dstep_d[t, :, 8:10, :], qs[:, :, :])
                    nc.sync.dma_start(dstep_d[t, :, 10:12, :], zc[:, :, :])

            if debug:
                nc.sync.dma_start(dgzr_d[:, :, :, :], gzr[:, :, :, :])
                nc.sync.dma_start(dgg_d[:, :, :, :], gg[:, :, :, :])
                nc.sync.dma_start(dh_d[:, :, :], hb[:, :, :])
            # ---- logits: AllGather h, then [B,VSC] = h @ embT_slice ----
            cc_in = ldram.tile([128, 2 * 32], bf16)
            nc.sync.dma_start(cc_in[:, :], hb[:, :, :])
            cc_out = ldram.tile([NCORES, 128, 2 * 32], bf16)
            nc.gpsimd.collective_compute(
                "AllGather",
                mybir.AluOpType.bypass,
                replica_groups=[list(range(NCORES))],
                ins=[cc_in.opt()],
                outs=[cc_out.opt()],
            )
            hall = const.tile([128, 2, NCORES, 32], bf16)  # [p, k, core, b]
            nc.sync.dma_start(
                hall[:, :, :, :],
                cc_out.rearrange("r p (k b) -> p k r b", k=2))

            nv = (VSC + 511) // 512
            for j in range(nv):
                v0 = j * 512
                vw = min(512, VSC - v0)
                for bt in range(2):
                    pl = pg1.tile([128, 512], f32, tag="pg")
                    for k in range(2):
                        nc.tensor.matmul(pl[:, :vw],
                                         hall[:, k, bt * 4:(bt + 1) * 4, :],
                                         embt_sb[:, k, v0:v0 + vw],
                                         start=(k == 0), stop=(k == 1))
                    lo = xtp.tile([128, 512], f32, tag="lo")
                    eng = (nc.scalar.copy,
                           nc.vector.tensor_copy)[(j * 2 + bt) % 2]
                    eng(lo[:, :vw], pl[:, :vw])
                    nc.sync.dma_start(
                        out_d[bt * 128:(bt + 1) * 128, v0:v0 + vw], lo[:, :vw])

    nc.compile()
    return nc


def _prep_inputs(input_ids, lengths, emb, w_ih, w_hh, b_ih, b_hh, with_bias):
    bfd = ml_dtypes.bfloat16
    emb32 = np.ascontiguousarray(emb.astype(np.float32))
    embbf = np.ascontiguousarray(emb32.astype(bfd))
    wih16 = w_ih.astype(bfd)
    whh16 = w_hh.astype(bfd)
    idm = np.eye(128, dtype=np.float32)
    if with_bias:
        biasf = (b_ih + b_hh).astype(np.float32).copy()
        biasf[2 * H:] = b_ih[2 * H:]      # h-candidate: b_ih only (pre r-mult)
        biasf = biasf.reshape(3 * H, 1)
        bhhg = b_hh[2 * H:].astype(np.float32).reshape(H, 1)

    in_maps = []
    for c in range(NCORES):
        bs = slice(c * BL, (c + 1) * BL)
        ids_c = np.ascontiguousarray(
            input_ids[bs].T.reshape(NTOK, 1).astype(np.int32))   # t-major
        mask_c = (np.arange(T)[:, None] >= lengths[bs][None, :])  # [T, BL]
        mask_c = np.ascontiguousarray(
            mask_c.reshape(1, NTOK).astype(bfd))
        v0 = c * VS
        embt_c = np.ascontiguousarray(emb32[v0:v0 + VSC].T.astype(bfd))
        m = {
            "ids": ids_c,
            "maskrow": mask_c,
            "embbf": embbf,
            "embt": embt_c,
            "wih": wih16,
            "whh": whh16,
            "idm": idm,
        }
        if with_bias:
            m["biasf"] = biasf
            m["bhhg"] = bhhg
        in_maps.append(m)
    return in_maps


def _run(in_maps, with_bias, trace=False, debug=False):
    from concourse.bass_utils import run_bass_kernel_spmd
    key = ("ncb" if with_bias else "nc") + ("d" if debug else "")
    if key not in _cache:
        _cache[key] = _build_nc(with_bias=with_bias, debug=debug)
    return run_bass_kernel_spmd(
        _cache[key], in_maps, core_ids=list(range(NCORES)), trace=trace)


def kernel(input_ids, lengths, emb, w_ih, w_hh, b_ih, b_hh, _trace=False):
    input_ids = np.asarray(input_ids)
    lengths = np.asarray(lengths)
    emb = np.asarray(emb, dtype=np.float32)
    w_ih = np.asarray(w_ih, dtype=np.float32)
    w_hh = np.asarray(w_hh, dtype=np.float32)
    b_ih = np.asarray(b_ih, dtype=np.float32)
    b_hh = np.asarray(b_hh, dtype=np.float32)
    with_bias = bool(np.any(b_ih != 0.0) or np.any(b_hh != 0.0))

    in_maps = _prep_inputs(input_ids, lengths, emb, w_ih, w_hh, b_ih, b_hh,
                           with_bias)
    res = _run(in_maps, with_bias, trace=_trace)
    outs = res.results if hasattr(res, "results") else res
    logits = np.empty((B, V + 1), np.float32)
    for c in range(NCORES):
        oc = outs[c]["out"].astype(np.float32)
        w = VSC if c == NCORES - 1 else VS
        logits[:, c * VS: c * VS + w] = oc[:, :w]
    if _trace:
        return logits, res
    return logits


# revision 15
# speedup vs baseline: 1.0759x; 1.0759x over previous
"""GRU4Rec Trainium2 kernel: 8-core SPMD.

Sharding: data-parallel over batch for embedding gather + input-gate matmuls +
GRU recurrence (32 sequences/core); vocab-sharded tied-embedding logits with an
on-device AllGather of the final hidden state.

Layout: "transposed" — hidden dim on partitions, batch on the free dim.
  - gates psum tile [128, slot, 32]: slots z0 z1 r0 r1 (zr bank) / g0 g1 (g bank)
  - per-step x-side z/r gates injected into PSUM via an identity-matmul with
    stop=True (keeps the serial elementwise chain off the Vector engine)
  - padding mask folded in as +BIG on the z-gate via a K=1 matmul of a 0/1 row
  - h kept fp32 (hf) with the fp32 update emitted AFTER the next step's
    matmuls (off the critical path); bf16 copy (hb) feeds the recurrence
  - phase-1 gathers read a bf16 copy of the embedding table and each chunk's
    work is spread across its 8 steps; PSUM->SBUF moves distributed over
    Scalar/Vector/GpSimd
  - logits: vocab slice preloaded to SBUF during the recurrence, bf16 output
    upconverted on host
"""

import numpy as np
import ml_dtypes

B, T, H, V = 256, 200, 256, 50000
NCORES = 8
BL = B // NCORES          # 32 sequences per core
NTOK = BL * T             # 6400 tokens per core
VS = 6250                 # vocab stride per core
VSC = VS + 1              # per-core logits width (overlap of 1, core 7 owns +1 row)
CHUNK_T = 8               # timesteps per phase-1 chunk
NCHUNK = T // CHUNK_T     # 25
CTOK = BL * CHUNK_T       # 256 tokens per chunk
BIGMASK = 60.0            # sigmoid(x + 60) == 1.0 in fp32

_cache = {}


def _build_nc(with_bias=False, debug=False):
    import concourse.bass as bass
    import concourse.mybir as mybir
    import concourse.tile as tile
    from concourse import bacc
    from concourse.bass import IndirectOffsetOnAxis

    f32 = mybir.dt.float32
    bf16 = mybir.dt.bfloat16
    i32 = mybir.dt.int32
    AF = mybir.ActivationFunctionType
    OP = mybir.AluOpType

    nc = bacc.Bacc(None, target_bir_lowering=False, debug=False, num_devices=NCORES)

    ids_d = nc.dram_tensor("ids", [NTOK, 1], i32, kind="ExternalInput")
    maskr_d = nc.dram_tensor("maskrow", [1, NTOK], bf16, kind="ExternalInput")
    embbf_d = nc.dram_tensor("embbf", [V + 1, H], bf16, kind="ExternalInput")
    embt_d = nc.dram_tensor("embt", [H, VSC], bf16, kind="ExternalInput")
    wih_d = nc.dram_tensor("wih", [H, 3 * H], bf16, kind="ExternalInput")
    whh_d = nc.dram_tensor("whh", [H, 3 * H], bf16, kind="ExternalInput")
    idm_d = nc.dram_tensor("idm", [128, 128], f32, kind="ExternalInput")
    if with_bias:
        biasf_d = nc.dram_tensor("biasf", [3 * H, 1], f32, kind="ExternalInput")
        bhhg_d = nc.dram_tensor("bhhg", [H, 1], f32, kind="ExternalInput")
    out_d = nc.dram_tensor("out", [B, VSC], f32, kind="ExternalOutput")
    if debug:
        dgzr_d = nc.dram_tensor("dgzr", [128, T, 4, 32], bf16, kind="ExternalOutput")
        dgg_d = nc.dram_tensor("dgg", [128, T, 2, 32], f32, kind="ExternalOutput")
        dh_d = nc.dram_tensor("dh", [128, 2, 32], bf16, kind="ExternalOutput")
        dxt_d = nc.dram_tensor("dxt", [128, 2, CTOK], bf16, kind="ExternalOutput")
        dstep_d = nc.dram_tensor("dstep", [2, 128, 12, 32], f32,
                                 kind="ExternalOutput")
        dpsum_d = nc.dram_tensor("dpsum", [128, 4, 32], f32,
                                 kind="ExternalOutput")

    with tile.TileContext(nc) as tc:
        with (
            tc.tile_pool(name="const", bufs=1) as const,
            tc.tile_pool(name="gstore", bufs=1) as gstore,
            tc.tile_pool(name="gin", bufs=3) as gin,
            tc.tile_pool(name="xtp", bufs=2) as xtp,
            tc.tile_pool(name="ew", bufs=2) as ew,
            tc.tile_pool(name="hst", bufs=2) as hst,
            tc.tile_pool(name="ldram", bufs=1, space="DRAM") as ldram,
            tc.tile_pool(name="ptr", bufs=2, space="PSUM") as ptr,
            tc.tile_pool(name="pg1", bufs=2, space="PSUM") as pg1,
            tc.tile_pool(name="pzr", bufs=2, space="PSUM") as pzr,
            tc.tile_pool(name="pgg", bufs=2, space="PSUM") as pgg,
        ):
            # ---- constants / weights ----
            wih_sb = const.tile([128, 2 * 3 * H], bf16)
            whh_sb = const.tile([128, 2 * 3 * H], bf16)
            for k in range(2):
                nc.sync.dma_start(wih_sb[:, k * 768:(k + 1) * 768],
                                  wih_d[k * 128:(k + 1) * 128, :])
                nc.sync.dma_start(whh_sb[:, k * 768:(k + 1) * 768],
                                  whh_d[k * 128:(k + 1) * 128, :])
            ident_f = const.tile([128, 128], f32)
            nc.sync.dma_start(ident_f[:, :], idm_d[:, :])
            ident = const.tile([128, 128], bf16)
            nc.vector.tensor_copy(ident[:, :], ident_f[:, :])
            wbig = const.tile([1, 128], bf16)
            nc.vector.memset(wbig[:, :], BIGMASK)
            maskr = const.tile([1, NTOK], bf16)
            nc.sync.dma_start(maskr[:, :], maskr_d[:, :])
            # logits vocab slice: preload during recurrence (25KB/partition)
            embt_sb = const.tile([128, 2, VSC], bf16)
            for k in range(2):
                nc.sync.dma_start(embt_sb[:, k, :],
                                  embt_d[k * 128:(k + 1) * 128, :])
            if with_bias:
                bias_sb = const.tile([128, 6], f32)
                nc.sync.dma_start(bias_sb[:, :],
                                  biasf_d.rearrange("(m p) o -> p (m o)", p=128))
                bhhg_sb = const.tile([128, 2], f32)
                nc.sync.dma_start(bhhg_sb[:, :],
                                  bhhg_d.rearrange("(g p) o -> p (g o)", p=128))

            # persistent gate stores
            gzr = gstore.tile([128, T, 4, 32], bf16)   # x-side z,r (+bias, +mask)
            gg = gstore.tile([128, T, 2, 32], f32)     # x-side h-candidate (+b_ih)

            # ---- phase-1, one chunk = 8 pieces spread over 8 steps ----
            # piece 0/1: gather 128 tokens (bf16); 2/3: transpose them;
            # 4/5/6: gate matmuls + PSUM->SBUF moves (2 gate slots per piece)
            chunk_state = {}
            deferred = []

            def emit_piece(c, j):
                tok0 = c * CTOK
                st = chunk_state.setdefault(c, {})
                if j in (0, 1):
                    idt = gin.tile([128, 1], i32, tag="idt")
                    nc.sync.dma_start(idt[:, :],
                                      ids_d[tok0 + j * 128: tok0 + (j + 1) * 128, :])
                    xg = gin.tile([128, H], bf16, tag="xg")
                    nc.gpsimd.indirect_dma_start(
                        out=xg[:, :], out_offset=None, in_=embbf_d[:, :],
                        in_offset=IndirectOffsetOnAxis(ap=idt[:, :1], axis=0))
                    st[j] = xg
                elif j in (2, 3):
                    tt = j - 2
                    if tt == 0:
                        st["xtc"] = xtp.tile([128, 2, CTOK], bf16, tag="xtc",
                                             name="xtc")
                    xg = st.pop(tt)
                    for hk in range(2):
                        pt = ptr.tile([128, 128], bf16, tag="pt")
                        nc.tensor.transpose(pt[:, :], xg[:, hk * 128:(hk + 1) * 128],
                                            ident[:, :])
                        dstx = st["xtc"][:, hk, tt * 128:(tt + 1) * 128]
                        if hk == 0:
                            nc.vector.tensor_copy(dstx, pt[:, :])
                        else:
                            nc.scalar.copy(dstx, pt[:, :])
                elif j in (4, 5, 6):
                    xtc = st["xtc"]
                    csl = slice(c * CHUNK_T, (c + 1) * CHUNK_T)
                    for mi, m in enumerate((2 * (j - 4), 2 * (j - 4) + 1)):
                        pg = pg1.tile([128, CHUNK_T, 32], f32, tag="pg")
                        for k in range(2):
                            nc.tensor.matmul(
                                pg[:, :, :],
                                wih_sb[:, k * 768 + m * 128: k * 768 + (m + 1) * 128],
                                xtc[:, k, :],
                                start=(k == 0), stop=(k == 1 and m >= 2))
                        if m < 2:  # z-gate slots: add BIGMASK * is_padded(token)
                            nc.tensor.matmul(
                                pg[:, :, :], wbig[:1, :],
                                maskr[:1, tok0: tok0 + CTOK],
                                start=False, stop=True)
                        dst = gzr[:, csl, m, :] if m < 4 else gg[:, csl, m - 4, :]
                        if with_bias:
                            nc.scalar.add(dst, pg[:, :, :], bias_sb[:, m:m + 1])
                        elif mi == 0:
                            nc.scalar.copy(dst, pg[:, :, :])
                        else:
                            nc.vector.tensor_copy(dst, pg[:, :, :])
                    if j == 6:
                        if debug and c == 0:
                            nc.sync.dma_start(dxt_d[:, :, :], xtc[:, :, :])
                        chunk_state.pop(c)

            # ---- recurrence ----
            hf = hst.tile([128, 2, 32], f32, tag="hf")
            hb = hst.tile([128, 2, 32], bf16, tag="hb")
            nc.vector.memset(hf[:, :, :], 0.0)
            nc.vector.memset(hb[:, :, :], 0.0)

            for c in range(2):
                for j in range(7):
                    emit_piece(c, j)
                    for fn in deferred:
                        fn()
                    deferred.clear()

            prev = None  # (us, ws, hf_prev) of step t-1, fp32 update deferred
            for t in range(T):
                cn = t // CHUNK_T + 2
                if cn < NCHUNK:
                    emit_piece(cn, t % CHUNK_T)

                przr = pzr.tile([128, 4, 32], f32, tag="przr")
                prg = pgg.tile([128, 2, 32], f32, tag="prg")
                for s in range(4):
                    for k in range(2):
                        nc.tensor.matmul(
                            przr[:, s, :],
                            whh_sb[:, k * 768 + s * 128: k * 768 + (s + 1) * 128],
                            hb[:, k, :], start=(k == 0), stop=False)
                    # x-side z/r gate (incl. mask) injected via identity-matmul
                    nc.tensor.matmul(przr[:, s, :], ident[:, :],
                                     gzr[:, t, s, :], start=False, stop=True)
                for s in range(2):
                    for k in range(2):
                        nc.tensor.matmul(
                            prg[:, s, :],
                            whh_sb[:, k * 768 + (4 + s) * 128: k * 768 + (5 + s) * 128],
                            hb[:, k, :], start=(k == 0), stop=(k == 1))

                # deferred fp32 h update for step t-1 (off the critical path:
                # emitted after this step's matmuls, needed only by us(t))
                if prev is not None:
                    hf = hst.tile([128, 2, 32], f32, tag="hf")
                    nc.vector.tensor_tensor(hf[:, :, :], prev[0], prev[1],
                                            op=OP.add)

                if debug and t == 1:
                    pcp = ew.tile([128, 4, 32], f32, tag="pcp")
                    nc.vector.tensor_copy(pcp[:, :, :], przr[:, :, :])
                    nc.sync.dma_start(dpsum_d[:, :, :], pcp[:, :, :])
                zr = ew.tile([128, 4, 32], f32, tag="zr")
                nc.scalar.activation(zr[:, :, :], przr[:, :, :], AF.Sigmoid)
                qs = ew.tile([128, 2, 32], f32, tag="qs")
                if with_bias:
                    for s in range(2):
                        nc.vector.scalar_tensor_tensor(
                            qs[:, s, :], prg[:, s, :], bhhg_sb[:, s:s + 1],
                            zr[:, 2 + s, :], op0=OP.add, op1=OP.mult)
                else:
                    nc.vector.tensor_tensor(qs[:, :, :], prg[:, :, :],
                                            zr[:, 2:4, :], op=OP.mult)
                q2 = ew.tile([128, 2, 32], f32, tag="q2")
                nc.vector.tensor_tensor(q2[:, :, :], qs[:, :, :], gg[:, t, :, :],
                                        op=OP.add)
                zc = ew.tile([128, 2, 32], f32, tag="zc")
                nc.vector.tensor_scalar(zc[:, :, :], zr[:, 0:2, :], -1.0, 1.0,
                                        OP.mult, OP.add)
                us = ew.tile([128, 2, 32], f32, tag="us")
                nc.vector.tensor_tensor(us[:, :, :], zr[:, 0:2, :], hf[:, :, :],
                                        op=OP.mult)
                hh = ew.tile([128, 2, 32], f32, tag="hh")
                nc.scalar.activation(hh[:, :, :], q2[:, :, :], AF.Tanh)
                ws = ew.tile([128, 2, 32], f32, tag="ws")
                nc.vector.tensor_tensor(ws[:, :, :], zc[:, :, :], hh[:, :, :],
                                        op=OP.mult)
                hb = hst.tile([128, 2, 32], bf16, tag="hb")
                nc.vector.tensor_tensor(hb[:, :, :], us[:, :, :], ws[:, :, :],
                                        op=OP.add)
                prev = (us[:, :, :], ws[:, :, :])
                if debug and t < 2:
                    hfd = ew.tile([128, 2, 32], f32, tag="hfd")
                    nc.gpsimd.tensor_tensor(hfd[:, :, :], us[:, :, :],
                                            ws[:, :, :], op=OP.add)
                    nc.sync.dma_start(dstep_d[t, :, 0:2, :], hfd[:, :, :])
                    nc.sync.dma_start(dstep_d[t, :, 2:6, :], zr[:, :, :])
                    nc.sync.dma_start(dstep_d[t, :, 6:8, :], hh[:, :, :])
                    nc.sync.dma_start(dstep_d[t, :, 8:10, :], qs[:, :, :])
                    nc.sync.dma_start(dstep_d[t, :, 10:12, :], zc[:, :, :])

            if debug:
                nc.sync.dma_start(dgzr_d[:, :, :, :], gzr[:, :, :, :])
                nc.sync.dma_start(dgg_d[:, :, :, :], gg[:, :, :, :])
                nc.sync.dma_start(dh_d[:, :, :], hb[:, :, :])
            # ---- logits: AllGather h, then [B,VSC] = h @ embT_slice ----
            cc_in = ldram.tile([128, 2 * 32], bf16)
            nc.sync.dma_start(cc_in[:, :], hb[:, :, :])
            cc_out = ldram.tile([NCORES, 128, 2 * 32], bf16)
            nc.gpsimd.collective_compute(
                "AllGather",
                mybir.AluOpType.bypass,
                replica_groups=[list(range(NCORES))],
                ins=[cc_in.opt()],
                outs=[cc_out.opt()],
            )
            hall = const.tile([128, 2, NCORES, 32], bf16)  # [p, k, core, b]
            nc.sync.dma_start(
                hall[:, :, :, :],
                cc_out.rearrange("r p (k b) -> p k r b", k=2))

            nv = (VSC + 511) // 512
            for j in range(nv):
                v0 = j * 512
                vw = min(512, VSC - v0)
                for bt in range(2):
                    pl = pg1.tile([128, 512], f32, tag="pg")
                    for k in range(2):
                        nc.tensor.matmul(pl[:, :vw],
                                         hall[:, k, bt * 4:(bt + 1) * 4, :],
                                         embt_sb[:, k, v0:v0 + vw],
                                         start=(k == 0), stop=(k == 1))
                    lo = xtp.tile([128, 512], f32, tag="lo")
                    eng = (nc.scalar.copy,
                           nc.vector.tensor_copy)[(j * 2 + bt) % 2]
                    eng(lo[:, :vw], pl[:, :vw])
                    nc.sync.dma_start(
                        out_d[bt * 128:(bt + 1) * 128, v0:v0 + vw], lo[:, :vw])

    nc.compile()
    return nc


def _prep_inputs(input_ids, lengths, emb, w_ih, w_hh, b_ih, b_hh, with_bias):
    bfd = ml_dtypes.bfloat16
    emb32 = np.ascontiguousarray(emb.astype(np.float32))
    embbf = np.ascontiguousarray(emb32.astype(bfd))
    wih16 = w_ih.astype(bfd)
    whh16 = w_hh.astype(bfd)
    idm = np.eye(128, dtype=np.float32)
    if with_bias:
        biasf = (b_ih + b_hh).astype(np.float32).copy()
        biasf[2 * H:] = b_ih[2 * H:]      # h-candidate: b_ih only (pre r-mult)
        biasf = biasf.reshape(3 * H, 1)
        bhhg = b_hh[2 * H:].astype(np.float32).reshape(H, 1)

    in_maps = []
    for c in range(NCORES):
        bs = slice(c * BL, (c + 1) * BL)
        ids_c = np.ascontiguousarray(
            input_ids[bs].T.reshape(NTOK, 1).astype(np.int32))   # t-major
        mask_c = (np.arange(T)[:, None] >= lengths[bs][None, :])  # [T, BL]
        mask_c = np.ascontiguousarray(
            mask_c.reshape(1, NTOK).astype(bfd))
        v0 = c * VS
        embt_c = np.ascontiguousarray(emb32[v0:v0 + VSC].T.astype(bfd))
        m = {
            "ids": ids_c,
            "maskrow": mask_c,
            "embbf": embbf,
            "embt": embt_c,
            "wih": wih16,
            "whh": whh16,
            "idm": idm,
        }
        if with_bias:
            m["biasf"] = biasf
            m["bhhg"] = bhhg
        in_maps.append(m)
    return in_maps


def _run(in_maps, with_bias, trace=False, debug=False):
    from concourse.bass_utils import run_bass_kernel_spmd
    key = ("ncb" if with_bias else "nc") + ("d" if debug else "")
    if key not in _cache:
        _cache[key] = _build_nc(with_bias=with_bias, debug=debug)
    return run_bass_kernel_spmd(
        _cache[key], in_maps, core_ids=list(range(NCORES)), trace=trace)


def kernel(input_ids, lengths, emb, w_ih, w_hh, b_ih, b_hh, _trace=False):
    input_ids = np.asarray(input_ids)
    lengths = np.asarray(lengths)
    emb = np.asarray(emb, dtype=np.float32)
    w_ih = np.asarray(w_ih, dtype=np.float32)
    w_hh = np.asarray(w_hh, dtype=np.float32)
    b_ih = np.asarray(b_ih, dtype=np.float32)
    b_hh = np.asarray(b_hh, dtype=np.float32)
    with_bias = bool(np.any(b_ih != 0.0) or np.any(b_hh != 0.0))

    in_maps = _prep_inputs(input_ids, lengths, emb, w_ih, w_hh, b_ih, b_hh,
                           with_bias)
    res = _run(in_maps, with_bias, trace=_trace)
    outs = res.results if hasattr(res, "results") else res
    logits = np.empty((B, V + 1), np.float32)
    for c in range(NCORES):
        oc = outs[c]["out"].astype(np.float32)
        w = VSC if c == NCORES - 1 else VS
        logits[:, c * VS: c * VS + w] = oc[:, :w]
    if _trace:
        return logits, res
    return logits
